# revision 1
# baseline (speedup 1.0000x reference)
# Trainium2 Bass kernel for LocLoss: per-sample argmax over a 192x192 cls map,
# gather of loc values at the argmax position, smooth-L1 loss vs a
# center_rate-derived bias, mean-reduced.
#
# Sharding: pure data parallel, batch 256 -> 8 cores x 32 samples.
# Per-core layout: the 36864-element cls map of sample s is split into 4
# chunks of 48 rows; partition p = s*4 + ch holds chunk ch. One bulk
# reduce_max pass produces per-row maxes; everything after operates on tiny
# (32, k) tiles. loc is never read in bulk: the 2 needed values per sample
# are fetched with an indirect DMA gather at the computed (r, c).
import numpy as np
from contextlib import ExitStack

import concourse.bass as bass
import concourse.bacc as bacc
import concourse.mybir as mybir
import concourse.tile as tile

B = 256
NCORES = 8
BP = B // NCORES          # 32 samples per core
H = W = 192
MAP = H * W               # 36864
NCHUNK = 4                # chunks per sample -> 128 partitions
ROWS_PER_PART = H // NCHUNK   # 48
CHUNK = ROWS_PER_PART * W     # 9216
NSLICE = 6                # streaming slices of the bulk cls load
SL_ROWS = ROWS_PER_PART // NSLICE   # 8 rows per partition per slice
SL_ELEMS = SL_ROWS * W              # 1536

F32 = mybir.dt.float32
U32 = mybir.dt.uint32
I32 = mybir.dt.int32
ALU = mybir.AluOpType


def build_program(with_dbg=False):
    nc = bacc.Bacc("TRN2", target_bir_lowering=False, debug=False, num_devices=NCORES)

    # cls as (rows, W): row index = s*192 + r, contiguous with host (32, 36864)
    cls_d = nc.dram_tensor("cls", [BP * H, W], F32, kind="ExternalInput")
    # host-shuffled copy in (ch, s, chunk) order: the bulk load for partition
    # p = ch*BP + s streams sequential DRAM with 9-36KB descriptors
    cls_shuf_d = nc.dram_tensor("cls_shuf", [128, CHUNK], F32, kind="ExternalInput")
    # loc host-transposed to (s, pos, ch) so both channel values at a map
    # position are adjacent: one indirect-gather index per sample fetches 2
    # contiguous elements (HW DGE gathers use one index per partition).
    loc_d = nc.dram_tensor("loc", [BP * MAP * 2 // 2048, 2048], F32,
                           kind="ExternalInput")
    cr_d = nc.dram_tensor("cr", [BP, 2], F32, kind="ExternalInput")
    loss_d = nc.dram_tensor("loss", [BP, 2], F32, kind="ExternalOutput")
    dbg_d = (nc.dram_tensor("dbg", [BP, 8], F32, kind="ExternalOutput")
             if with_dbg else None)

    with tile.TileContext(nc) as tc:
        with ExitStack() as ctx:
            const = ctx.enter_context(tc.tile_pool(name="const", bufs=1))
            stream = ctx.enter_context(tc.tile_pool(name="stream", bufs=3))
            small = ctx.enter_context(tc.tile_pool(name="small", bufs=1))

            cls_view = cls_shuf_d[:]  # (128, 9216), p = ch*BP + s

            # --- bulk pass: per-(partition, row) max -> (128, 48)
            # SWDGE (gpsimd) DMAs round-robin across 8 queues -> 16 SDMA
            # engines; HWDGE queues all pin to the same 4 engines. Slices
            # shrink toward the end so the final reduce trails the last
            # (tiny, low-latency HWDGE) load by well under 1us.
            slice_rows = [10, 10, 10, 10, 7, 1]
            row_max = const.tile([128, ROWS_PER_PART], F32)
            r0 = 0
            for i, nrows in enumerate(slice_rows):
                eng = nc.sync if i == len(slice_rows) - 1 else nc.gpsimd
                t = stream.tile([128, nrows * W], F32, tag=f"cls_slice{i}")
                eng.dma_start(t[:], cls_view[:, r0 * W:(r0 + nrows) * W])
                nc.vector.reduce_max(
                    row_max[:, r0:r0 + nrows],
                    t[:].rearrange("p (a c) -> p a c", c=W),
                    axis=mybir.AxisListType.X,
                )
                r0 += nrows

            # --- per-sample row maxes: rowT[s, r] over all 192 global rows
            rowT = small.tile([BP, H], F32)
            rowt_engines = [nc.sync, nc.scalar, nc.gpsimd, nc.sync]
            for ch in range(NCHUNK):
                rowt_engines[ch].dma_start(
                    rowT[:, ch * ROWS_PER_PART:(ch + 1) * ROWS_PER_PART],
                    row_max[ch * BP:(ch + 1) * BP, :],
                )

            m8 = small.tile([BP, 8], F32)
            ri8 = small.tile([BP, 8], U32)
            nc.vector.max(out=m8[:], in_=rowT[:])
            nc.vector.max_index(out=ri8[:], in_max=m8[:], in_values=rowT[:])

            r_f = small.tile([BP, 1], F32)
            nc.vector.tensor_copy(r_f[:], ri8[:, 0:1])

            # global row index into cls_d: s*192 + r
            s192_i = small.tile([BP, 1], I32)
            nc.gpsimd.iota(s192_i[:], pattern=[[1, 1]], base=0, channel_multiplier=H)
            s192_f = small.tile([BP, 1], F32)
            nc.vector.tensor_copy(s192_f[:], s192_i[:])
            rowidx_f = small.tile([BP, 1], F32)
            nc.vector.tensor_tensor(rowidx_f[:], r_f[:], s192_f[:], op=ALU.add)
            rowidx_u = small.tile([BP, 1], U32)
            nc.vector.tensor_copy(rowidx_u[:], rowidx_f[:])

            # gather each sample's winning row (192 f32) from DRAM
            rows_t = small.tile([BP, W], F32)
            nc.gpsimd.indirect_dma_start(
                out=rows_t[:],
                out_offset=None,
                in_=cls_d[:],
                in_offset=bass.IndirectOffsetOnAxis(ap=rowidx_u[:, 0:1], axis=0),
            )

            rm8 = small.tile([BP, 8], F32)
            ci8 = small.tile([BP, 8], U32)
            nc.vector.max(out=rm8[:], in_=rows_t[:])
            nc.vector.max_index(out=ci8[:], in_max=rm8[:], in_values=rows_t[:])
            c_f = small.tile([BP, 1], F32)
            nc.vector.tensor_copy(c_f[:], ci8[:, 0:1])

            # loc flat element offsets: off[s, ch] = s*73728 + ch*36864 + r*192 + c
            # element offset = 2*(s*36864 + r*192 + c); iota gives 2*s,
            # scaled by 36864 (iota pattern steps are int16-bound)
            base_i = small.tile([BP, 1], I32)
            nc.gpsimd.iota(base_i[:], pattern=[[1, 1]], base=0,
                           channel_multiplier=2)
            base_f = small.tile([BP, 1], F32)
            nc.vector.tensor_copy(base_f[:], base_i[:])
            nc.vector.tensor_scalar_mul(base_f[:], base_f[:], float(MAP))

            rc_f = small.tile([BP, 1], F32)
            nc.vector.tensor_scalar(rc_f[:], r_f[:], float(W), c_f[:, 0:1],
                                    op0=ALU.mult, op1=ALU.add)
            off_f = small.tile([BP, 1], F32)
            nc.vector.scalar_tensor_tensor(off_f[:], rc_f[:], 2.0, base_f[:],
                                           op0=ALU.mult, op1=ALU.add)
            off_u = small.tile([BP, 1], U32)
            nc.vector.tensor_copy(off_u[:], off_f[:])

            loc_pos = small.tile([BP, 2], F32)
            nc.gpsimd.indirect_dma_start(
                out=loc_pos[:],
                out_offset=None,
                in_=loc_d[:],
                in_offset=bass.IndirectOffsetOnAxis(ap=off_u[:, 0:1], axis=1),
            )

            # bias = center_rate*191 - [r, c]
            cr_t = small.tile([BP, 2], F32)
            nc.sync.dma_start(cr_t[:], cr_d[:])
            rc2 = small.tile([BP, 2], F32)
            nc.vector.tensor_copy(rc2[:, 0:1], r_f[:])
            nc.vector.tensor_copy(rc2[:, 1:2], c_f[:])
            bias = small.tile([BP, 2], F32)
            nc.vector.tensor_scalar(bias[:], cr_t[:], float(H - 1), None,
                                    op0=ALU.mult)
            nc.vector.tensor_tensor(bias[:], bias[:], rc2[:], op=ALU.subtract)

            # smooth L1 (beta=1)
            diff = small.tile([BP, 2], F32)
            nc.vector.tensor_tensor(diff[:], loc_pos[:], bias[:], op=ALU.subtract)
            ad = small.tile([BP, 2], F32)
            nc.scalar.activation(ad[:], diff[:], mybir.ActivationFunctionType.Abs)
            quad = small.tile([BP, 2], F32)
            nc.vector.scalar_tensor_tensor(quad[:], ad[:], 0.5, ad[:],
                                           op0=ALU.mult, op1=ALU.mult)
            lin = small.tile([BP, 2], F32)
            nc.vector.tensor_scalar_add(lin[:], ad[:], -0.5)
            mlt = small.tile([BP, 2], F32)
            nc.vector.tensor_scalar(mlt[:], ad[:], 1.0, None, op0=ALU.is_lt)
            # lval = lin + mlt*(quad - lin)
            tsel = small.tile([BP, 2], F32)
            nc.vector.tensor_tensor(tsel[:], quad[:], lin[:], op=ALU.subtract)
            nc.vector.tensor_tensor(tsel[:], mlt[:], tsel[:], op=ALU.mult)
            lval = small.tile([BP, 2], F32)
            nc.vector.tensor_tensor(lval[:], lin[:], tsel[:], op=ALU.add)

            nc.sync.dma_start(loss_d[:], lval[:])

            if with_dbg:
                dbg = small.tile([BP, 8], F32)
                nc.vector.tensor_copy(dbg[:, 0:1], m8[:, 0:1])
                nc.vector.tensor_copy(dbg[:, 1:2], r_f[:])
                nc.vector.tensor_copy(dbg[:, 2:3], c_f[:])
                nc.vector.tensor_copy(dbg[:, 3:5], loc_pos[:])
                nc.vector.tensor_copy(dbg[:, 5:7], bias[:])
                nc.vector.tensor_copy(dbg[:, 7:8], rm8[:, 0:1])
                nc.sync.dma_start(dbg_d[:], dbg[:])

    nc.compile()
    return nc


_NC_CACHE = None


def _get_program():
    global _NC_CACHE
    if _NC_CACHE is None:
        _NC_CACHE = build_program()
    return _NC_CACHE


def make_in_maps(cls_input, loc_input, center_rate):
    cls = np.ascontiguousarray(np.asarray(cls_input, dtype=np.float32)).reshape(
        NCORES, BP * H, W)
    cls_shuf = np.ascontiguousarray(
        cls.reshape(NCORES, BP, NCHUNK, CHUNK).transpose(0, 2, 1, 3)).reshape(
        NCORES, 128, CHUNK)
    loc = np.asarray(loc_input, dtype=np.float32).reshape(B, 2, MAP)
    loc = np.ascontiguousarray(loc.transpose(0, 2, 1)).reshape(
        NCORES, BP * MAP * 2 // 2048, 2048)
    cr = np.ascontiguousarray(np.asarray(center_rate, dtype=np.float32)).reshape(
        NCORES, BP, 2)
    return [
        {"cls": cls[c], "cls_shuf": cls_shuf[c], "loc": loc[c], "cr": cr[c]}
        for c in range(NCORES)
    ]


def kernel(cls_input, loc_input, center_rate, _trace=False, _results_out=None):
    from concourse.bass_utils import run_bass_kernel_spmd

    nc = _get_program()
    in_maps = make_in_maps(cls_input, loc_input, center_rate)
    res = run_bass_kernel_spmd(nc, in_maps, list(range(NCORES)), trace=_trace)
    if _results_out is not None:
        _results_out.append(res)
    losses = np.concatenate([r["loss"] for r in res.results], axis=0)  # (256, 2)
    return np.float32(np.mean(losses, dtype=np.float64))



# revision 21
# speedup vs baseline: 1.1034x; 1.1034x over previous
# Trainium2 Bass kernel for LocLoss: per-sample argmax over a 192x192 cls map,
# gather of loc values at the argmax position, smooth-L1 loss vs a
# center_rate-derived bias, mean-reduced.
#
# Sharding: pure data parallel, batch 256 -> 8 cores x 32 samples.
# Per-core layout: partition p = 4*s + ch holds chunk ch (48 rows) of sample
# s's 192x192 map -- a pure host reshape, no shuffle. The bulk load streams
# over the sync HWDGE queue in 8 in-order slices with the per-row max reduced
# on Vector as each slice lands. The tail then runs at chunk-candidate
# granularity (one candidate per partition, all offsets affine in p):
#   max/max_index over the 48 row-maxes -> winning row rr per chunk
#   two concurrent indirect gathers, both keyed on rr only:
#     cls winning row (128,192) and loc winning row pair (128,384)
#   max_index on the gathered cls row -> column cc
#   one-hot dot extracts the two loc values; smooth-L1 on-device.
# Output per core: (128, 2) = [chunk max value, lossY+lossX]; the host picks
# the best of each sample's 4 chunk candidates (the global argmax) and means.
import numpy as np
from contextlib import ExitStack

import concourse.bass as bass
import concourse.bacc as bacc
import concourse.mybir as mybir
import concourse.tile as tile

B = 256
NCORES = 8
BP = B // NCORES          # 32 samples per core
H = W = 192
MAP = H * W               # 36864
NCHUNK = 4                # chunks per sample -> 128 partitions
ROWS_PER_PART = H // NCHUNK   # 48
CHUNK = ROWS_PER_PART * W     # 9216

# bulk slices (rows per partition); must sum to ROWS_PER_PART. Sized so the
# vector reduce chain trails the last DMA byte by ~1us.
SLICE_ROWS = [10, 9, 8, 7, 6, 5, 2, 1]
assert sum(SLICE_ROWS) == ROWS_PER_PART

F32 = mybir.dt.float32
I32 = mybir.dt.int32
U32 = mybir.dt.uint32
ALU = mybir.AluOpType
AX = mybir.AxisListType


def build_program(with_dbg=False, stage=6):
    nc = bacc.Bacc("TRN2", target_bir_lowering=False, debug=False, num_devices=NCORES)

    # cls as rows of 192: row index for the gather is 48*p + rr
    cls_d = nc.dram_tensor("cls", [128 * ROWS_PER_PART, W], F32,
                           kind="ExternalInput")
    # loc host-transposed to (s, r, c, ch) rows of 384: row index of the
    # winning row pair is 48*p + rr -- the same index as the cls row gather.
    loc_d = nc.dram_tensor("loc", [128 * ROWS_PER_PART, 2 * W], F32,
                           kind="ExternalInput")
    # aux[p] = [48*p, 191*crY[s] - 48*(p%4), 191*crX[s], 0] (host-folded)
    aux_d = nc.dram_tensor("aux", [128, 4], F32, kind="ExternalInput")
    loss_d = nc.dram_tensor("loss", [128, 2], F32, kind="ExternalOutput")
    dbg_d = (nc.dram_tensor("dbg", [128, 8], F32, kind="ExternalOutput")
             if with_dbg else None)

    cls_rows = cls_d[:].rearrange("(p r) c -> p (r c)", p=128)  # (128, 9216)

    with tile.TileContext(nc) as tc:
        with ExitStack() as ctx:
            pool = ctx.enter_context(tc.tile_pool(name="p", bufs=1))

            aux = pool.tile([128, 4], F32, tag="aux")
            nc.scalar.dma_start(aux[:], aux_d[:])

            # col iota 0..191 as f32, used by the one-hot select
            iota_i = pool.tile([128, W], I32, tag="iotai")
            iota_f = pool.tile([128, W], F32, tag="iotaf")
            nc.gpsimd.iota(iota_i[:], pattern=[[1, W]], base=0,
                           channel_multiplier=0)
            nc.vector.tensor_copy(iota_f[:], iota_i[:])

            row_max = pool.tile([128, ROWS_PER_PART], F32, tag="rowmax")

            # --- bulk: stream cls in slices over the sync HWDGE queue,
            # reducing each slice's rows on Vector as it lands.
            r0 = 0
            for i, nrows in enumerate(SLICE_ROWS):
                t = pool.tile([128, nrows * W], F32, tag=f"s{i}")
                nc.sync.dma_start(t[:], cls_rows[:, r0 * W:(r0 + nrows) * W])
                nc.vector.reduce_max(
                    row_max[:, r0:r0 + nrows],
                    t[:].rearrange("p (a c) -> p a c", c=W),
                    axis=AX.X,
                )
                r0 += nrows

            if stage <= 1:
                nc.sync.dma_start(loss_d[:], row_max[:, 0:2])

            if stage >= 2:
                # --- per-chunk argmax row
                m8 = pool.tile([128, 8], F32, tag="m8")
                ri8 = pool.tile([128, 8], U32, tag="ri8")
                nc.vector.max(out=m8[:], in_=row_max[:])
                nc.vector.max_index(out=ri8[:], in_max=m8[:], in_values=row_max[:])

                rcf = pool.tile([128, 2], F32, tag="rcf")   # [rr, cc] as f32
                nc.vector.tensor_copy(rcf[:, 0:1], ri8[:, 0:1])

                # shared gather row index: 48*p + rr
                rowf = pool.tile([128, 1], F32, tag="rowf")
                nc.vector.scalar_tensor_tensor(rowf[:], rcf[:, 0:1], 1.0,
                                               aux[:, 0:1], op0=ALU.mult,
                                               op1=ALU.add)
                rowu = pool.tile([128, 1], U32, tag="rowu")
                nc.vector.tensor_copy(rowu[:], rowf[:])

                if stage <= 2:
                    nc.sync.dma_start(loss_d[:], rcf[:])

            if stage >= 3:
                # --- two concurrent gathers, both keyed on the winning row
                rows_t = pool.tile([128, W], F32, tag="rows")
                nc.gpsimd.indirect_dma_start(
                    out=rows_t[:],
                    out_offset=None,
                    in_=cls_d[:],
                    in_offset=bass.IndirectOffsetOnAxis(ap=rowu[:, 0:1], axis=0),
                )
                if stage <= 3:
                    nc.sync.dma_start(loss_d[:], rows_t[:, 0:2])

            if stage >= 4:
                locrow = pool.tile([128, 2 * W], F32, tag="locrow")
                nc.gpsimd.indirect_dma_start(
                    out=locrow[:],
                    out_offset=None,
                    in_=loc_d[:],
                    in_offset=bass.IndirectOffsetOnAxis(ap=rowu[:, 0:1], axis=0),
                )
                if stage <= 4:
                    nc.sync.dma_start(loss_d[:], locrow[:, 0:2])

            if stage >= 5:
                cand = pool.tile([128, 2], F32, tag="cand")  # [val, loss sum]
                nc.vector.tensor_copy(cand[:, 0:1], m8[:, 0:1])  # in gathers

                # --- column of the chunk max within the gathered row
                # (max_index faults if a searched value is absent, so re-max
                # over the gathered row rather than reusing m8[1:].)
                rm8 = pool.tile([128, 8], F32, tag="rm8")
                ci8 = pool.tile([128, 8], U32, tag="ci8")
                nc.vector.max(out=rm8[:], in_=rows_t[:])
                nc.vector.max_index(out=ci8[:], in_max=rm8[:], in_values=rows_t[:])
                nc.vector.tensor_copy(rcf[:, 1:2], ci8[:, 0:1])

                # --- one-hot dot: loc values at column cc
                # (TensorScalarPtr only allows arithmetic combos like
                # (mult-imm, add-AP); comparisons must be immediate-only.)
                onehot = pool.tile([128, W], F32, tag="onehot")
                nc.vector.tensor_scalar(onehot[:], iota_f[:], -1.0, rcf[:, 1:2],
                                        op0=ALU.mult, op1=ALU.add)  # cc - iota
                nc.vector.tensor_tensor(onehot[:], onehot[:], onehot[:],
                                        op=ALU.mult)                # squared
                nc.vector.tensor_scalar(onehot[:], onehot[:], 0.5, None,
                                        op0=ALU.is_lt)              # one-hot
                scr = pool.tile([128, W], F32, tag="scr")
                scrx = pool.tile([128, W], F32, tag="scrx")
                loc_pos = pool.tile([128, 2], F32, tag="locp")
                oh3 = onehot[:].rearrange("p (a one) -> p a one", one=1)
                scr3 = scr[:].rearrange("p (a one) -> p a one", one=1)
                scrx3 = scrx[:].rearrange("p (a one) -> p a one", one=1)
                lr3 = locrow[:].rearrange("p (a t) -> p a t", t=2)
                nc.vector.tensor_tensor(scr3, oh3, lr3[:, :, 0:1], op=ALU.mult)
                nc.vector.tensor_tensor(scrx3, oh3, lr3[:, :, 1:2], op=ALU.mult)
                nc.vector.tensor_reduce(loc_pos[:, 0:1], scr[:], axis=AX.X,
                                        op=ALU.add)
                nc.vector.tensor_reduce(loc_pos[:, 1:2], scrx[:], axis=AX.X,
                                        op=ALU.add)
                if stage <= 5:
                    nc.sync.dma_start(loss_d[:], loc_pos[:])

            if stage >= 6:
                # d = loc - (191*cr - [r_map, c]) = (loc - aux[:,2:4]) + [rr,cc]
                d2 = pool.tile([128, 2], F32, tag="d2")
                nc.vector.tensor_tensor(d2[:], loc_pos[:], aux[:, 1:3],
                                        op=ALU.subtract)
                nc.vector.tensor_tensor(d2[:], d2[:], rcf[:], op=ALU.add)
                # smooth L1 (beta=1): a=|d|; h=min(a,1); loss = h*(a - 0.5h)
                u2 = pool.tile([128, 2], F32, tag="u2")
                nc.vector.tensor_tensor(u2[:], d2[:], d2[:], op=ALU.mult)
                a2 = pool.tile([128, 2], F32, tag="a2")
                nc.scalar.sqrt(a2[:], u2[:])
                h2 = pool.tile([128, 2], F32, tag="h2")
                nc.vector.tensor_scalar_min(h2[:], a2[:], 1.0)
                t2 = pool.tile([128, 2], F32, tag="t2")
                nc.vector.scalar_tensor_tensor(t2[:], h2[:], -0.5, a2[:],
                                               op0=ALU.mult, op1=ALU.add)
                l2 = pool.tile([128, 2], F32, tag="l2")
                nc.vector.tensor_tensor(l2[:], h2[:], t2[:], op=ALU.mult)
                nc.vector.tensor_tensor(cand[:, 1:2], l2[:, 0:1], l2[:, 1:2],
                                        op=ALU.add)

                nc.sync.dma_start(loss_d[:], cand[:])

            if with_dbg:
                dbg = pool.tile([128, 8], F32, tag="dbg")
                nc.vector.tensor_copy(dbg[:, 0:1], m8[:, 0:1])
                nc.vector.tensor_copy(dbg[:, 1:3], rcf[:])
                nc.vector.tensor_copy(dbg[:, 3:4], off_f[:])
                nc.vector.tensor_copy(dbg[:, 4:6], loc_pos[:])
                nc.vector.tensor_copy(dbg[:, 6:8], l2[:])
                nc.sync.dma_start(dbg_d[:], dbg[:])

    nc.compile()
    return nc


_NC_CACHE = {}


def _get_program(with_dbg=False):
    if with_dbg not in _NC_CACHE:
        _NC_CACHE[with_dbg] = build_program(with_dbg)
    return _NC_CACHE[with_dbg]


_P = np.arange(128)
_AUX_C0 = (18432.0 * _P).astype(np.float32)        # exact in f32 (< 2^24)
_AUX_C1 = (48.0 * _P).astype(np.float32)
_AUX_ROW = (48.0 * (_P % 4)).astype(np.float32)


def make_in_maps(cls_input, loc_input, center_rate):
    # p = 4*s + ch: pure reshape, rows 48*ch..48*ch+47 of sample s -> part p
    cls = np.ascontiguousarray(np.asarray(cls_input, dtype=np.float32)).reshape(
        NCORES, 128 * ROWS_PER_PART, W)
    loc = np.asarray(loc_input, dtype=np.float32).reshape(B, 2, MAP)
    loc = np.ascontiguousarray(loc.transpose(0, 2, 1)).reshape(
        NCORES, 128 * ROWS_PER_PART, 2 * W)
    cr = np.asarray(center_rate, dtype=np.float32).reshape(NCORES, BP, 2)
    crr = np.repeat(cr, NCHUNK, axis=1)            # (NCORES, 128, 2)
    aux = np.zeros((NCORES, 128, 4), dtype=np.float32)
    aux[:, :, 0] = _AUX_C1
    aux[:, :, 1] = np.float32(191.0) * crr[:, :, 0] - _AUX_ROW
    aux[:, :, 2] = np.float32(191.0) * crr[:, :, 1]
    return [
        {"cls": cls[c], "loc": loc[c], "aux": aux[c]}
        for c in range(NCORES)
    ]


def kernel(cls_input, loc_input, center_rate, _trace=False, _results_out=None,
           _dbg=False):
    from concourse.bass_utils import run_bass_kernel_spmd

    nc = _get_program(_dbg)
    in_maps = make_in_maps(cls_input, loc_input, center_rate)
    res = run_bass_kernel_spmd(nc, in_maps, list(range(NCORES)), trace=_trace)
    if _results_out is not None:
        _results_out.append(res)
    out = np.stack([r["loss"] for r in res.results], axis=0)  # (8, 128, 2)
    vals = out[:, :, 0].reshape(B, NCHUNK)
    ls = out[:, :, 1].reshape(B, NCHUNK)
    sel = np.argmax(vals, axis=1)
    loss_sum = ls[np.arange(B), sel]
    return np.float32(np.sum(loss_sum, dtype=np.float64) / (2 * B))
